# revision 1
# baseline (speedup 1.0000x reference)
"""RNN-T decoder + joint network Trainium2 kernel (8-core SPMD).

Sharding: data-parallel over batch B=8 -> one batch element per core.

Single fused pipeline per core (everything on-device):
  z_enc = W_enc @ hs^T + b_enc                      (PE, fp16)
  Xg0 = W_ih0 @ eys^T + b                           (PE, fp16, batched)
  main loop over u (layer-1 lags layer-0 by LAG=8):
    - 2-layer LSTM steps: h-matvecs on PE (fp16; 1-col matvecs are
      LDWEIGHTS-rate-bound at ~27ns per 128x128 tile), gate math on
      ScalarE/VectorE/GpSimd with gates host-permuted to [i,f,o,g]
      so one sigmoid covers i,f,o.  Layer-1 x-part batched per 8.
    - z_dec = W_dec @ h1 batched per 8 steps         (PE, fp16)
    - joint tanh(z_enc + z_dec[u]) per u             (ScalarE bias port)
    - joint (T*U,512) @ W_out^T tiles interleaved between LSTM steps
      (<=2-3 row tiles of 128 per step) so the serial recurrence
      latency hides under the big fp16 matmul streams
    - PSUM eviction + b_out add -> fp16 on VectorE
    - contiguous DMA of fp16 (T*U, 1000) rows to HBM
  Input DMAs are issued in strict arrival-need order (biases first --
  they gate the xg0/zenc evictions -- then W_ih0, W_hh0, ...).
Host stages layouts (transpose/cast/embedding gather), then upcasts
fp16 -> fp32 and reassembles (B, T, U, ODIM).
"""

import numpy as np

B, T, U = 8, 200, 50
E = 512          # EPROJS == DUNITS == EMB == JOINT
ODIM = 1000
BLANK = 0
NG = 16          # gate-dim tiles of 128 (4*DUNITS / 128)
KT = 4           # contraction tiles of 128 (E / 128)
NT = T * U       # joint rows, u-major: row = u*T + t
VJ = 500         # vocab split per PSUM bank
LAG = 8          # layer-1 lag behind layer-0

_CACHE = {}


def _install_tile_patch():
    """This walrus build rejects >1 sync wait on one instruction; spread the
    Tile epilogue drain's waits across single-wait NoOp carriers."""
    import concourse.mybir as mybir
    import concourse.tile as tile_mod
    from concourse.vector_clock import ScopedClock

    if getattr(tile_mod.TileContext, "_drain_patched", False):
        return

    def _drain_and_barrier(self, tick_clock, wait_clock):
        drain_inst = self.nc.sync.drain()
        wait_clock.add_sem_waits(
            drain_inst.ins, ScopedClock({None: tick_clock.global_clock})
        )
        si = drain_inst.ins.sync_info
        if si is not None and si.on_wait and len(si.on_wait) > 1:
            waits = list(si.on_wait)
            ups = list(si.on_update) if si.on_update else []
            drain_inst.ins.sync_info = mybir.SyncInfo(
                on_wait=waits[:1], on_update=ups
            )
            for w in waits[1:]:
                nop = self.nc.sync.nop()
                nop.ins.sync_info = mybir.SyncInfo(on_wait=[w], on_update=[])
        self.nc.all_engine_barrier()
        assert self.sems is not None
        popped = self.nc._tile_sem_poison_stack.pop()
        assert popped is self._sem_poison
        self.nc.clear_and_free_semaphores(list(self.sems.allocated().values()))
        self.nc.all_engine_barrier()

    tile_mod.TileContext._drain_and_barrier = _drain_and_barrier
    tile_mod.TileContext._drain_patched = True


def _split_multi_waits(nc):
    """This walrus build allows one sync wait per instruction. Hoist excess
    waits onto single-wait NoOp carriers directly before the instruction on
    the same engine (program order on the sequencer preserves semantics)."""
    import concourse.mybir as mybir

    n_new = 0
    for fn in nc.m.functions:
        for blk in fn.blocks:
            ins = blk.instructions
            out = []
            dirty = False
            for inst in ins:
                si = inst.sync_info
                if si is not None and si.on_wait and len(si.on_wait) > 1:
                    waits = list(si.on_wait)
                    ups = list(si.on_update) if si.on_update else []
                    for w in waits[:-1]:
                        nop = mybir.InstNoOp(
                            name=f"{inst.name}_w{n_new}", ins=[], outs=[]
                        )
                        n_new += 1
                        nop.engine = inst.engine
                        nop.sync_info = mybir.SyncInfo(on_wait=[w], on_update=[])
                        out.append(nop)
                    inst.sync_info = mybir.SyncInfo(
                        on_wait=[waits[-1]], on_update=ups
                    )
                    dirty = True
                out.append(inst)
            if dirty:
                blk.instructions = out
    return n_new


def _build_nc():
    import concourse.bass as bass
    import concourse.mybir as mybir
    import concourse.tile as tile

    _install_tile_patch()
    f16, f32 = mybir.dt.float16, mybir.dt.float32
    f8 = mybir.dt.float8e4
    Sig = mybir.ActivationFunctionType.Sigmoid
    Tanh = mybir.ActivationFunctionType.Tanh

    nc = bass.Bass()
    d_hsT = nc.dram_tensor("hsT", [E, T], f16, kind="ExternalInput")
    d_eysT = nc.dram_tensor("eysT", [E, U], f16, kind="ExternalInput")
    d_wih0 = nc.dram_tensor("wih0T8", [E, 4 * E], f8, kind="ExternalInput")
    d_whh0 = nc.dram_tensor("whh0T8", [E, 4 * E], f8, kind="ExternalInput")
    d_wih1 = nc.dram_tensor("wih1T8", [E, 4 * E], f8, kind="ExternalInput")
    d_whh1 = nc.dram_tensor("whh1T8", [E, 4 * E], f8, kind="ExternalInput")
    d_wenc = nc.dram_tensor("wencT", [E, E], f16, kind="ExternalInput")
    d_wdec = nc.dram_tensor("wdecT", [E, E], f16, kind="ExternalInput")
    d_wout = nc.dram_tensor("woutT", [E, ODIM], f16, kind="ExternalInput")
    d_bg0 = nc.dram_tensor("bg0", [128, NG], f32, kind="ExternalInput")
    d_bg1 = nc.dram_tensor("bg1", [128, NG], f32, kind="ExternalInput")
    d_benc = nc.dram_tensor("bencT", [128, KT], f32, kind="ExternalInput")
    d_bout = nc.dram_tensor("boutB", [128, ODIM], f32, kind="ExternalInput")
    d_out = nc.dram_tensor("out", [NT, ODIM], f16, kind="ExternalOutput")

    with tile.TileContext(nc) as tc:
        with (
            tc.tile_pool(name="wp", bufs=1) as wp,
            tc.tile_pool(name="sp", bufs=1) as sp,
            tc.tile_pool(name="gp", bufs=3) as gp,
            tc.tile_pool(name="op", bufs=3) as op,
            tc.tile_pool(name="pp", bufs=1, space="PSUM") as pp,
        ):
            # ---- persistent weight tiles --------------------------------
            def load4(dram, width, dt=f16, name="", eng=None):
                ts = []
                for k in range(KT):
                    t = wp.tile([128, width], dt, tag=f"{name}{k}", name=f"{name}{k}")
                    (eng or nc.sync).dma_start(t[:], dram[k * 128:(k + 1) * 128, :])
                    ts.append(t)
                return ts

            # single DMA queue, strict arrival-need order: tiny biases
            # first (they gate the xg0/zenc evictions), then xg0 weights
            # (they gate the whole PE stream), then the recurrence weights
            bg0 = wp.tile([128, NG], f32, tag="bg0", name="bg0")
            nc.sync.dma_start(bg0[:], d_bg0[:])
            bg1 = wp.tile([128, NG], f32, tag="bg1", name="bg1")
            nc.sync.dma_start(bg1[:], d_bg1[:])
            benc = wp.tile([128, KT], f32, tag="benc", name="benc")
            nc.sync.dma_start(benc[:], d_benc[:])
            eysT = load4(d_eysT, U, name="eysT")
            wih0 = load4(d_wih0, 4 * E, dt=f8, name="wih0")
            whh0 = load4(d_whh0, 4 * E, dt=f8, name="whh0")
            wenc = load4(d_wenc, E, name="wenc")
            hsT = load4(d_hsT, T, name="hsT")
            bout = wp.tile([128, ODIM], f32, tag="bout", name="bout")
            nc.sync.dma_start(bout[:], d_bout[:])
            wih1 = load4(d_wih1, 4 * E, dt=f8, name="wih1")
            whh1 = load4(d_whh1, 4 * E, dt=f8, name="whh1")
            wdec = load4(d_wdec, E, name="wdec")
            wout = load4(d_wout, ODIM, name="wout")

            # ---- state tiles --------------------------------------------
            zenc = [sp.tile([128, T], f32, tag=f"zenc{k}", name=f"zenc{k}") for k in range(KT)]
            zdec = [sp.tile([128, U], f32, tag=f"zdec{k}", name=f"zdec{k}") for k in range(KT)]
            tmpT = [sp.tile([128, NT], f16, tag=f"tmpT{k}", name=f"tmpT{k}") for k in range(KT)]
            xg0 = sp.tile([128, NG * U], f32, tag="xg0", name="xg0")   # [mt*U + u]
            xg1 = sp.tile([128, NG * U], f32, tag="xg1", name="xg1")
            h0h = sp.tile([128, U * KT], f16, tag="h0h", name="h0h")   # [u*KT + c]
            h1h = sp.tile([128, U * KT], f16, tag="h1h", name="h1h")
            c0 = sp.tile([128, KT], f32, tag="c0", name="c0")
            c1 = sp.tile([128, KT], f32, tag="c1", name="c1")
            nc.vector.memset(c0[:], 0.0)
            nc.vector.memset(c1[:], 0.0)

            xg0r = xg0.rearrange("p (m u) -> p m u", u=U)
            xg1r = xg1.rearrange("p (m u) -> p m u", u=U)
            h0r = h0h.rearrange("p (u c) -> p u c", c=KT)
            h1r = h1h.rearrange("p (u c) -> p u c", c=KT)

            # ---- z_enc = W_enc @ hsT + b_enc (emitted per-tile as early
            # ---- main-loop filler while the joint has no work yet) ------
            def zenc_tile(mt):
                ps = pp.tile([128, VJ], f32, tag="js", bufs=4, name="js")
                for k in range(KT):
                    nc.tensor.matmul(
                        ps[:, :T], wenc[k][:, mt * 128:(mt + 1) * 128],
                        hsT[k][:], start=(k == 0), stop=(k == KT - 1),
                    )
                rec("vector", nc.vector.tensor_scalar_add(
                    zenc[mt][:], ps[:, :T], benc[:, mt:mt + 1]
                ))

            # ---- Xg0 = W_ih0 @ eysT + (b_ih0 + b_hh0) -------------------
            for mt in range(NG):
                ps = pp.tile([128, VJ], f32, tag="js", bufs=4, name="js")
                for k in range(KT):
                    nc.tensor.matmul(
                        ps[:, :U], wih0[k][:, mt * 128:(mt + 1) * 128],
                        eysT[k][:], start=(k == 0), stop=(k == KT - 1),
                    )
                nc.vector.tensor_scalar_add(
                    xg0[:, mt * U:(mt + 1) * U], ps[:, :U], bg0[:, mt:mt + 1]
                )

            # ---- LSTM cell step (engine split as in baseline) -----------
            lstm_ord = {"vector": [], "scalar": [], "gpsimd": []}

            def rec(eng, bi):
                lstm_ord[eng].append(bi)
                return bi

            def lstm_step(u, whh, xgr, hr, c, hist, aux, tg_):
                # gate tile order is host-permuted to [i, f, o, g] so one
                # sigmoid covers i,f,o and one tanh covers g
                aux_name = "vector" if aux is nc.vector else "gpsimd"
                pifo = pp.tile([128, 12], f32, tag="pifo" + tg_, bufs=1,
                               name="pifo")
                pg = pp.tile([128, KT], f32, tag="pg" + tg_, bufs=1,
                             name="pg")
                if u > 0:
                    for mt in range(12):
                        for k in range(KT):
                            nc.tensor.matmul(
                                pifo[:, mt:mt + 1],
                                whh[k][:, mt * 128:(mt + 1) * 128],
                                hr[:, u - 1, k:k + 1],
                                start=(k == 0), stop=(k == KT - 1),
                            )
                gifo = gp.tile([128, 12], f32, tag="gifo" + tg_, name="gifo")
                gg = gp.tile([128, KT], f32, tag="gg" + tg_, name="gg")
                if u > 0:
                    rec("vector",
                        nc.vector.tensor_add(gifo[:], pifo[:], xgr[:, 0:12, u]))
                    for mt in range(12, NG):
                        for k in range(KT):
                            nc.tensor.matmul(
                                pg[:, mt - 12:mt - 11],
                                whh[k][:, mt * 128:(mt + 1) * 128],
                                hr[:, u - 1, k:k + 1],
                                start=(k == 0), stop=(k == KT - 1),
                            )
                    rec("vector",
                        nc.vector.tensor_add(gg[:], pg[:], xgr[:, 12:NG, u]))
                else:
                    rec("vector", nc.vector.tensor_copy(gifo[:], xgr[:, 0:12, u]))
                    rec("vector", nc.vector.tensor_copy(gg[:], xgr[:, 12:NG, u]))
                sifo = gp.tile([128, 12], f32, tag="sifo" + tg_, name="sifo")
                tg = gp.tile([128, KT], f32, tag="tg" + tg_, name="tg")
                rec("scalar", nc.scalar.activation(sifo[:], gifo[:], Sig))
                rec("scalar", nc.scalar.activation(tg[:], gg[:], Tanh))
                t1 = gp.tile([128, KT], f32, tag="t1" + tg_, name="t1")
                t2 = gp.tile([128, KT], f32, tag="t2" + tg_, name="t2")
                rec(aux_name, aux.tensor_mul(t1[:], sifo[:, 4:8], c[:]))
                rec(aux_name, aux.tensor_mul(t2[:], sifo[:, 0:4], tg[:]))
                rec(aux_name, aux.tensor_add(c[:], t1[:], t2[:]))
                tc_ = gp.tile([128, KT], f32, tag="tc" + tg_, name="tc")
                rec("scalar", nc.scalar.activation(tc_[:], c[:], Tanh))
                rec(aux_name,
                    aux.tensor_mul(hist[:, u * KT:(u + 1) * KT], sifo[:, 8:12],
                                   tc_[:]))

            def xg1_block(lo, hi):
                for mt in range(NG):
                    ps = pp.tile([128, VJ], f32, tag="js", bufs=4, name="js")
                    for k in range(KT):
                        nc.tensor.matmul(
                            ps[:, :hi - lo],
                            wih1[k][:, mt * 128:(mt + 1) * 128],
                            h0r[:, lo:hi, k], start=(k == 0),
                            stop=(k == KT - 1),
                        )
                    rec("vector", nc.vector.tensor_scalar_add(
                        xg1r[:, mt, lo:hi], ps[:, :hi - lo], bg1[:, mt:mt + 1]
                    ))

            def zdec_block(lo, hi):
                for mt in range(KT):
                    ps = pp.tile([128, VJ], f32, tag="js", bufs=4, name="js")
                    for k in range(KT):
                        nc.tensor.matmul(
                            ps[:, :hi - lo], wdec[k][:, mt * 128:(mt + 1) * 128],
                            h1r[:, lo:hi, k], start=(k == 0), stop=(k == KT - 1),
                        )
                    rec("vector",
                        nc.vector.tensor_copy(zdec[mt][:, lo:hi], ps[:, :hi - lo]))

            # ---- joint tile: 128 rows of tanh'd tmpT @ W_out + b_out ----
            n_m = (NT + 127) // 128
            ev_flip = [0]

            def joint_rows(m):
                rows = min(128, NT - m * 128)
                ps0 = pp.tile([128, VJ], f32, tag="js", bufs=4, name="js")
                ps1 = pp.tile([128, VJ], f32, tag="js", bufs=4, name="js")
                for k in range(KT):
                    lhs = tmpT[k][:, m * 128:m * 128 + rows]
                    nc.tensor.matmul(ps0[:rows, :], lhs, wout[k][:, 0:VJ],
                                     start=(k == 0), stop=(k == KT - 1))
                    nc.tensor.matmul(ps1[:rows, :], lhs, wout[k][:, VJ:ODIM],
                                     start=(k == 0), stop=(k == KT - 1))
                osb = op.tile([128, ODIM], f16, tag="osb", name="osb")
                rec("vector",
                    nc.vector.tensor_add(osb[:rows, 0:VJ], ps0[:rows, :],
                                         bout[:rows, 0:VJ]))
                rec("vector",
                    nc.vector.tensor_add(osb[:rows, VJ:ODIM], ps1[:rows, :],
                                         bout[:rows, VJ:ODIM]))
                nc.sync.dma_start(d_out[m * 128:m * 128 + rows, :],
                                  osb[:rows, :])

            def tanh_rows(u):
                for k in range(KT):
                    rec("scalar", nc.scalar.activation(
                        tmpT[k][:, u * T:(u + 1) * T], zenc[k][:], Tanh,
                        bias=zdec[k][:, u:u + 1],
                    ))

            # ---- fused main loop ----------------------------------------
            m_done = 0
            zt_done = 0
            sc_m = nc.enter_named_scope("MAIN", False)
            for t in range(U + LAG):
                if t < U:
                    lstm_step(t, whh0, xg0r, h0r, c0, h0h, nc.vector, "0")
                    if 2 <= t < 2 + KT:
                        zenc_tile(t - 2)
                    if t % LAG == LAG - 1:
                        xg1_block(t - LAG + 1, t + 1)
                    elif t == U - 1 and U % LAG:
                        xg1_block(U - U % LAG, U)
                if t >= LAG:
                    u1 = t - LAG
                    lstm_step(u1, whh1, xg1r, h1r, c1, h1h, nc.gpsimd, "1")
                    if u1 == 3:
                        # one-time early mini-block so joint tiles exist
                        # before the first full block at u1=7
                        zdec_block(0, 4)
                        for uu in range(0, 4):
                            tanh_rows(uu)
                        zt_done = 4
                    elif u1 % LAG == LAG - 1 or u1 == U - 1:
                        lo = (u1 // LAG) * LAG
                        zdec_block(max(lo, zt_done), u1 + 1)
                        for uu in range(max(lo, zt_done), u1 + 1):
                            tanh_rows(uu)
                        zt_done = u1 + 1
                # emit a couple of joint row-tiles per step (3 when behind)
                # to bound the PE-stream delay seen by the recurrence
                avail = min((zt_done * T) // 128, n_m)
                cap = 2 + (avail - m_done > 6)
                emitted = 0
                while emitted < cap and m_done < avail:
                    joint_rows(m_done)
                    m_done += 1
                    emitted += 1
            while m_done < n_m:
                joint_rows(m_done)
                m_done += 1
            nc.leave_named_scope("MAIN", sc_m, False)

            from concourse.tile import add_dep_helper
            for eng, seq in lstm_ord.items():
                for a, b in zip(seq[1:], seq[:-1]):
                    add_dep_helper(a.ins, b.ins, sync=False,
                                   reason="lstm stream order")

    _split_multi_waits(nc)
    return nc


_PERM = np.concatenate([np.arange(0, 512), np.arange(512, 1024),
                        np.arange(1536, 2048), np.arange(1024, 1536)])


def _stage(inputs):
    import ml_dtypes
    f16 = np.float16
    f8 = ml_dtypes.float8_e4m3fn
    g = lambda k: np.asarray(inputs[k], dtype=np.float32)
    hs = g("hs_pad")
    ys = np.asarray(inputs["ys_in_pad"]).astype(np.int64)
    emb0 = g("emb").copy()
    emb0[BLANK] = 0.0
    bg0 = np.ascontiguousarray(
        (g("b_ih_0") + g("b_hh_0"))[_PERM].reshape(NG, 128).T)
    bg1 = np.ascontiguousarray(
        (g("b_ih_1") + g("b_hh_1"))[_PERM].reshape(NG, 128).T)
    benc = np.ascontiguousarray(g("b_enc").reshape(KT, 128).T)
    bout = np.ascontiguousarray(
        np.broadcast_to(g("b_out"), (128, ODIM)))
    shared = {
        "wih0T8": np.ascontiguousarray(g("W_ih_0")[_PERM].T.astype(f8)),
        "whh0T8": np.ascontiguousarray(g("W_hh_0")[_PERM].T.astype(f8)),
        "wih1T8": np.ascontiguousarray(g("W_ih_1")[_PERM].T.astype(f8)),
        "whh1T8": np.ascontiguousarray(g("W_hh_1")[_PERM].T.astype(f8)),
        "wencT": np.ascontiguousarray(g("W_enc").T.astype(f16)),
        "wdecT": np.ascontiguousarray(g("W_dec").T.astype(f16)),
        "woutT": np.ascontiguousarray(g("W_out").T.astype(f16)),
        "bg0": bg0, "bg1": bg1, "bencT": benc, "boutB": bout,
    }
    in_maps = []
    for b in range(B):
        m = dict(shared)
        m["hsT"] = np.ascontiguousarray(hs[b].T.astype(f16))
        m["eysT"] = np.ascontiguousarray(emb0[ys[b]].T.astype(f16))
        in_maps.append(m)
    return in_maps


def run(inputs, trace=False, ret_res=False):
    from concourse.bass_utils import run_bass_kernel_spmd

    if "nc" not in _CACHE:
        _CACHE["nc"] = _build_nc()
    nc = _CACHE["nc"]
    in_maps = _stage(inputs)
    res = run_bass_kernel_spmd(nc, in_maps, core_ids=list(range(B)),
                               trace=trace)
    _CACHE["last_res"] = res
    out = np.empty((B, T, U, ODIM), np.float32)
    for b in range(B):
        out[b] = res.results[b]["out"].astype(np.float32).reshape(
            U, T, ODIM).transpose(1, 0, 2)
    return out, res.exec_time_ns


def kernel(**inputs) -> np.ndarray:
    out, _ = run(inputs, trace=False)
    return out



# revision 2
# speedup vs baseline: 2.1555x; 2.1555x over previous
"""RNN-T decoder + joint network Trainium2 kernel (8-core SPMD).

Sharding: data-parallel over batch B=8 -> one batch element per core.

Single fused pipeline per core (everything on-device):
  z_enc = W_enc @ hs^T + b_enc                      (PE, fp16)
  Xg0 = W_ih0 @ eys^T + b                           (PE, fp16, batched)
  main loop over u (layer-1 lags layer-0 by LAG=8):
    - 2-layer LSTM steps: h-matvecs on PE (fp16; 1-col matvecs are
      LDWEIGHTS-rate-bound at ~27ns per 128x128 tile), gate math on
      ScalarE/VectorE/GpSimd with gates host-permuted to [i,f,o,g]
      so one sigmoid covers i,f,o.  Layer-1 x-part batched per 8.
    - z_dec = W_dec @ h1 batched per 8 steps         (PE, fp16)
    - joint tanh(z_enc + z_dec[u]) per u             (ScalarE bias port)
    - joint (T*U,512) @ W_out^T tiles interleaved between LSTM steps
      (<=2-3 row tiles of 128 per step) so the serial recurrence
      latency hides under the big fp16 matmul streams
    - PSUM eviction + b_out add -> fp16 on VectorE
    - contiguous DMA of fp16 (T*U, 1000) rows to HBM
  Input DMAs are issued in strict arrival-need order (biases first --
  they gate the xg0/zenc evictions -- then W_ih0, W_hh0, ...).
Host stages layouts (transpose/cast/embedding gather), then upcasts
fp16 -> fp32 and reassembles (B, T, U, ODIM).
"""

import numpy as np

B, T, U = 8, 200, 50
E = 512          # EPROJS == DUNITS == EMB == JOINT
ODIM = 1000
BLANK = 0
NG = 16          # gate-dim tiles of 128 (4*DUNITS / 128)
KT = 4           # contraction tiles of 128 (E / 128)
NT = T * U       # joint rows, u-major: row = u*T + t
VJ = 500         # vocab split per PSUM bank
LAG = 8          # layer-1 lag behind layer-0

_CACHE = {}


def _install_tile_patch():
    """This walrus build rejects >1 sync wait on one instruction; spread the
    Tile epilogue drain's waits across single-wait NoOp carriers."""
    import concourse.mybir as mybir
    import concourse.tile as tile_mod
    from concourse.vector_clock import ScopedClock

    if getattr(tile_mod.TileContext, "_drain_patched", False):
        return

    def _drain_and_barrier(self, tick_clock, wait_clock):
        drain_inst = self.nc.sync.drain()
        wait_clock.add_sem_waits(
            drain_inst.ins, ScopedClock({None: tick_clock.global_clock})
        )
        si = drain_inst.ins.sync_info
        if si is not None and si.on_wait and len(si.on_wait) > 1:
            waits = list(si.on_wait)
            ups = list(si.on_update) if si.on_update else []
            drain_inst.ins.sync_info = mybir.SyncInfo(
                on_wait=waits[:1], on_update=ups
            )
            for w in waits[1:]:
                nop = self.nc.sync.nop()
                nop.ins.sync_info = mybir.SyncInfo(on_wait=[w], on_update=[])
        self.nc.all_engine_barrier()
        assert self.sems is not None
        popped = self.nc._tile_sem_poison_stack.pop()
        assert popped is self._sem_poison
        self.nc.clear_and_free_semaphores(list(self.sems.allocated().values()))
        self.nc.all_engine_barrier()

    tile_mod.TileContext._drain_and_barrier = _drain_and_barrier
    tile_mod.TileContext._drain_patched = True


def _split_multi_waits(nc):
    """This walrus build allows one sync wait per instruction. Hoist excess
    waits onto single-wait NoOp carriers directly before the instruction on
    the same engine (program order on the sequencer preserves semantics)."""
    import concourse.mybir as mybir

    n_new = 0
    for fn in nc.m.functions:
        for blk in fn.blocks:
            ins = blk.instructions
            out = []
            dirty = False
            for inst in ins:
                si = inst.sync_info
                if si is not None and si.on_wait and len(si.on_wait) > 1:
                    waits = list(si.on_wait)
                    ups = list(si.on_update) if si.on_update else []
                    for w in waits[:-1]:
                        nop = mybir.InstNoOp(
                            name=f"{inst.name}_w{n_new}", ins=[], outs=[]
                        )
                        n_new += 1
                        nop.engine = inst.engine
                        nop.sync_info = mybir.SyncInfo(on_wait=[w], on_update=[])
                        out.append(nop)
                    inst.sync_info = mybir.SyncInfo(
                        on_wait=[waits[-1]], on_update=ups
                    )
                    dirty = True
                out.append(inst)
            if dirty:
                blk.instructions = out
    return n_new


def _build_nc():
    import concourse.bass as bass
    import concourse.mybir as mybir
    import concourse.tile as tile

    _install_tile_patch()
    f16, f32 = mybir.dt.float16, mybir.dt.float32
    f8 = mybir.dt.float8e4
    Sig = mybir.ActivationFunctionType.Sigmoid
    Tanh = mybir.ActivationFunctionType.Tanh

    nc = bass.Bass()
    d_hsT = nc.dram_tensor("hsT", [E, T], f16, kind="ExternalInput")
    d_eysT = nc.dram_tensor("eysT", [E, U], f16, kind="ExternalInput")
    d_wih0 = nc.dram_tensor("wih0T8", [E, 4 * E], f8, kind="ExternalInput")
    d_whh0 = nc.dram_tensor("whh0T8", [E, 4 * E], f8, kind="ExternalInput")
    d_wih1 = nc.dram_tensor("wih1T8", [E, 4 * E], f8, kind="ExternalInput")
    d_whh1 = nc.dram_tensor("whh1T8", [E, 4 * E], f8, kind="ExternalInput")
    d_wenc = nc.dram_tensor("wencT", [E, E], f16, kind="ExternalInput")
    d_wdec = nc.dram_tensor("wdecT", [E, E], f16, kind="ExternalInput")
    d_wout = nc.dram_tensor("woutT", [E, ODIM], f16, kind="ExternalInput")
    d_bg0 = nc.dram_tensor("bg0", [128, NG], f32, kind="ExternalInput")
    d_bg1 = nc.dram_tensor("bg1", [128, NG], f32, kind="ExternalInput")
    d_benc = nc.dram_tensor("bencT", [128, KT], f32, kind="ExternalInput")
    d_bout = nc.dram_tensor("boutB", [128, ODIM], f32, kind="ExternalInput")
    d_out = nc.dram_tensor("out", [NT, ODIM], f16, kind="ExternalOutput")

    with tile.TileContext(nc) as tc:
        with (
            tc.tile_pool(name="wp", bufs=1) as wp,
            tc.tile_pool(name="sp", bufs=1) as sp,
            tc.tile_pool(name="gp", bufs=3) as gp,
            tc.tile_pool(name="op", bufs=3) as op,
            tc.tile_pool(name="pp", bufs=1, space="PSUM") as pp,
        ):
            # ---- persistent weight tiles --------------------------------
            def load4(dram, width, dt=f16, name="", eng=None):
                ts = []
                for k in range(KT):
                    t = wp.tile([128, width], dt, tag=f"{name}{k}", name=f"{name}{k}")
                    (eng or nc.sync).dma_start(t[:], dram[k * 128:(k + 1) * 128, :])
                    ts.append(t)
                return ts

            # single DMA queue, strict arrival-need order: tiny biases
            # first (they gate the xg0/zenc evictions), then xg0 weights
            # (they gate the whole PE stream), then the recurrence weights
            bg0 = wp.tile([128, NG], f32, tag="bg0", name="bg0")
            nc.sync.dma_start(bg0[:], d_bg0[:])
            bg1 = wp.tile([128, NG], f32, tag="bg1", name="bg1")
            nc.sync.dma_start(bg1[:], d_bg1[:])
            benc = wp.tile([128, KT], f32, tag="benc", name="benc")
            nc.sync.dma_start(benc[:], d_benc[:])
            eysT = load4(d_eysT, U, name="eysT")
            wih0 = load4(d_wih0, 4 * E, dt=f8, name="wih0")
            whh0 = load4(d_whh0, 4 * E, dt=f8, name="whh0")
            wenc = load4(d_wenc, E, name="wenc")
            hsT = load4(d_hsT, T, name="hsT")
            bout = wp.tile([128, ODIM], f32, tag="bout", name="bout")
            nc.sync.dma_start(bout[:], d_bout[:])
            wih1 = load4(d_wih1, 4 * E, dt=f8, name="wih1")
            whh1 = load4(d_whh1, 4 * E, dt=f8, name="whh1")
            wdec = load4(d_wdec, E, name="wdec")
            wout = load4(d_wout, ODIM, name="wout")

            # ---- state tiles --------------------------------------------
            zenc = [sp.tile([128, T], f32, tag=f"zenc{k}", name=f"zenc{k}") for k in range(KT)]
            zdec = [sp.tile([128, U], f32, tag=f"zdec{k}", name=f"zdec{k}") for k in range(KT)]
            tmpT = [sp.tile([128, NT], f16, tag=f"tmpT{k}", name=f"tmpT{k}") for k in range(KT)]
            xg0 = sp.tile([128, NG * U], f32, tag="xg0", name="xg0")   # [mt*U + u]
            xg1 = sp.tile([128, NG * U], f32, tag="xg1", name="xg1")
            h0h = sp.tile([128, U * KT], f16, tag="h0h", name="h0h")   # [u*KT + c]
            h1h = sp.tile([128, U * KT], f16, tag="h1h", name="h1h")
            c0 = sp.tile([128, KT], f32, tag="c0", name="c0")
            c1 = sp.tile([128, KT], f32, tag="c1", name="c1")
            nc.vector.memset(c0[:], 0.0)
            nc.vector.memset(c1[:], 0.0)

            xg0r = xg0.rearrange("p (m u) -> p m u", u=U)
            xg1r = xg1.rearrange("p (m u) -> p m u", u=U)
            h0r = h0h.rearrange("p (u c) -> p u c", c=KT)
            h1r = h1h.rearrange("p (u c) -> p u c", c=KT)

            # ---- z_enc = W_enc @ hsT + b_enc (emitted per-tile as early
            # ---- main-loop filler while the joint has no work yet) ------
            def zenc_tile(mt):
                ps = pp.tile([128, VJ], f32, tag="js", bufs=4, name="js")
                for k in range(KT):
                    nc.tensor.matmul(
                        ps[:, :T], wenc[k][:, mt * 128:(mt + 1) * 128],
                        hsT[k][:], start=(k == 0), stop=(k == KT - 1),
                    )
                rec("vector", nc.vector.tensor_scalar_add(
                    zenc[mt][:], ps[:, :T], benc[:, mt:mt + 1]
                ))

            # ---- Xg0 = W_ih0 @ eysT + (b_ih0 + b_hh0) -------------------
            for mt in range(NG):
                ps = pp.tile([128, VJ], f32, tag="js", bufs=4, name="js")
                for k in range(KT):
                    nc.tensor.matmul(
                        ps[:, :U], wih0[k][:, mt * 128:(mt + 1) * 128],
                        eysT[k][:], start=(k == 0), stop=(k == KT - 1),
                    )
                nc.vector.tensor_scalar_add(
                    xg0[:, mt * U:(mt + 1) * U], ps[:, :U], bg0[:, mt:mt + 1]
                )

            # ---- LSTM cell step (engine split as in baseline) -----------
            lstm_ord = {"vector": [], "scalar": [], "gpsimd": []}

            def rec(eng, bi):
                lstm_ord[eng].append(bi)
                return bi

            def lstm_step(u, whh, xgr, hr, c, hist, aux, tg_):
                # gate tile order is host-permuted to [i, f, o, g] so one
                # sigmoid covers i,f,o and one tanh covers g
                aux_name = "vector" if aux is nc.vector else "gpsimd"
                pifo = pp.tile([128, 12], f32, tag="pifo" + tg_, bufs=1,
                               name="pifo")
                pg = pp.tile([128, KT], f32, tag="pg" + tg_, bufs=1,
                             name="pg")
                if u > 0:
                    for mt in range(12):
                        for k in range(KT):
                            nc.tensor.matmul(
                                pifo[:, mt:mt + 1],
                                whh[k][:, mt * 128:(mt + 1) * 128],
                                hr[:, u - 1, k:k + 1],
                                start=(k == 0), stop=(k == KT - 1),
                            )
                gifo = gp.tile([128, 12], f32, tag="gifo" + tg_, name="gifo")
                gg = gp.tile([128, KT], f32, tag="gg" + tg_, name="gg")
                if u > 0:
                    rec("vector",
                        nc.vector.tensor_add(gifo[:], pifo[:], xgr[:, 0:12, u]))
                    for mt in range(12, NG):
                        for k in range(KT):
                            nc.tensor.matmul(
                                pg[:, mt - 12:mt - 11],
                                whh[k][:, mt * 128:(mt + 1) * 128],
                                hr[:, u - 1, k:k + 1],
                                start=(k == 0), stop=(k == KT - 1),
                            )
                    rec("vector",
                        nc.vector.tensor_add(gg[:], pg[:], xgr[:, 12:NG, u]))
                else:
                    rec("vector", nc.vector.tensor_copy(gifo[:], xgr[:, 0:12, u]))
                    rec("vector", nc.vector.tensor_copy(gg[:], xgr[:, 12:NG, u]))
                sifo = gp.tile([128, 12], f32, tag="sifo" + tg_, name="sifo")
                tg = gp.tile([128, KT], f32, tag="tg" + tg_, name="tg")
                rec("scalar", nc.scalar.activation(sifo[:], gifo[:], Sig))
                rec("scalar", nc.scalar.activation(tg[:], gg[:], Tanh))
                t1 = gp.tile([128, KT], f32, tag="t1" + tg_, name="t1")
                t2 = gp.tile([128, KT], f32, tag="t2" + tg_, name="t2")
                rec(aux_name, aux.tensor_mul(t1[:], sifo[:, 4:8], c[:]))
                rec(aux_name, aux.tensor_mul(t2[:], sifo[:, 0:4], tg[:]))
                rec(aux_name, aux.tensor_add(c[:], t1[:], t2[:]))
                tc_ = gp.tile([128, KT], f32, tag="tc" + tg_, name="tc")
                rec("scalar", nc.scalar.activation(tc_[:], c[:], Tanh))
                rec(aux_name,
                    aux.tensor_mul(hist[:, u * KT:(u + 1) * KT], sifo[:, 8:12],
                                   tc_[:]))

            def xg1_block(lo, hi):
                for mt in range(NG):
                    ps = pp.tile([128, VJ], f32, tag="js", bufs=4, name="js")
                    for k in range(KT):
                        nc.tensor.matmul(
                            ps[:, :hi - lo],
                            wih1[k][:, mt * 128:(mt + 1) * 128],
                            h0r[:, lo:hi, k], start=(k == 0),
                            stop=(k == KT - 1),
                        )
                    rec("vector", nc.vector.tensor_scalar_add(
                        xg1r[:, mt, lo:hi], ps[:, :hi - lo], bg1[:, mt:mt + 1]
                    ))

            def zdec_block(lo, hi):
                for mt in range(KT):
                    ps = pp.tile([128, VJ], f32, tag="js", bufs=4, name="js")
                    for k in range(KT):
                        nc.tensor.matmul(
                            ps[:, :hi - lo], wdec[k][:, mt * 128:(mt + 1) * 128],
                            h1r[:, lo:hi, k], start=(k == 0), stop=(k == KT - 1),
                        )
                    rec("vector",
                        nc.vector.tensor_copy(zdec[mt][:, lo:hi], ps[:, :hi - lo]))

            # ---- joint tile: 128 rows of tanh'd tmpT @ W_out + b_out ----
            n_m = (NT + 127) // 128
            ev_flip = [0]

            def joint_rows(m):
                rows = min(128, NT - m * 128)
                ps0 = pp.tile([128, VJ], f32, tag="js", bufs=4, name="js")
                ps1 = pp.tile([128, VJ], f32, tag="js", bufs=4, name="js")
                for k in range(KT):
                    lhs = tmpT[k][:, m * 128:m * 128 + rows]
                    nc.tensor.matmul(ps0[:rows, :], lhs, wout[k][:, 0:VJ],
                                     start=(k == 0), stop=(k == KT - 1))
                    nc.tensor.matmul(ps1[:rows, :], lhs, wout[k][:, VJ:ODIM],
                                     start=(k == 0), stop=(k == KT - 1))
                osb = op.tile([128, ODIM], f16, tag="osb", name="osb")
                rec("vector",
                    nc.vector.tensor_add(osb[:rows, 0:VJ], ps0[:rows, :],
                                         bout[:rows, 0:VJ]))
                rec("vector",
                    nc.vector.tensor_add(osb[:rows, VJ:ODIM], ps1[:rows, :],
                                         bout[:rows, VJ:ODIM]))
                nc.sync.dma_start(d_out[m * 128:m * 128 + rows, :],
                                  osb[:rows, :])

            def tanh_rows(u):
                for k in range(KT):
                    rec("scalar", nc.scalar.activation(
                        tmpT[k][:, u * T:(u + 1) * T], zenc[k][:], Tanh,
                        bias=zdec[k][:, u:u + 1],
                    ))

            # ---- fused main loop ----------------------------------------
            m_done = 0
            zt_done = 0
            sc_m = nc.enter_named_scope("MAIN", False)
            for t in range(U + LAG):
                if t < U:
                    lstm_step(t, whh0, xg0r, h0r, c0, h0h, nc.vector, "0")
                    if 2 <= t < 2 + KT:
                        zenc_tile(t - 2)
                    if t % LAG == LAG - 1:
                        xg1_block(t - LAG + 1, t + 1)
                    elif t == U - 1 and U % LAG:
                        xg1_block(U - U % LAG, U)
                if t >= LAG:
                    u1 = t - LAG
                    lstm_step(u1, whh1, xg1r, h1r, c1, h1h, nc.gpsimd, "1")
                    if u1 == 3:
                        # one-time early mini-block so joint tiles exist
                        # before the first full block at u1=7
                        zdec_block(0, 4)
                        for uu in range(0, 4):
                            tanh_rows(uu)
                        zt_done = 4
                    elif u1 % LAG == LAG - 1 or u1 == U - 1:
                        lo = (u1 // LAG) * LAG
                        zdec_block(max(lo, zt_done), u1 + 1)
                        for uu in range(max(lo, zt_done), u1 + 1):
                            tanh_rows(uu)
                        zt_done = u1 + 1
                # emit a couple of joint row-tiles per step (3 when behind)
                # to bound the PE-stream delay seen by the recurrence
                avail = min((zt_done * T) // 128, n_m)
                cap = 3 + (avail - m_done > 6)
                emitted = 0
                while emitted < cap and m_done < avail:
                    joint_rows(m_done)
                    m_done += 1
                    emitted += 1
            while m_done < n_m:
                joint_rows(m_done)
                m_done += 1
            nc.leave_named_scope("MAIN", sc_m, False)

            from concourse.tile import add_dep_helper
            for eng, seq in lstm_ord.items():
                for a, b in zip(seq[1:], seq[:-1]):
                    add_dep_helper(a.ins, b.ins, sync=False,
                                   reason="lstm stream order")

    _split_multi_waits(nc)
    return nc


_PERM = np.concatenate([np.arange(0, 512), np.arange(512, 1024),
                        np.arange(1536, 2048), np.arange(1024, 1536)])


def _stage(inputs):
    import ml_dtypes
    f16 = np.float16
    f8 = ml_dtypes.float8_e4m3fn
    g = lambda k: np.asarray(inputs[k], dtype=np.float32)
    hs = g("hs_pad")
    ys = np.asarray(inputs["ys_in_pad"]).astype(np.int64)
    emb0 = g("emb").copy()
    emb0[BLANK] = 0.0
    bg0 = np.ascontiguousarray(
        (g("b_ih_0") + g("b_hh_0"))[_PERM].reshape(NG, 128).T)
    bg1 = np.ascontiguousarray(
        (g("b_ih_1") + g("b_hh_1"))[_PERM].reshape(NG, 128).T)
    benc = np.ascontiguousarray(g("b_enc").reshape(KT, 128).T)
    bout = np.ascontiguousarray(
        np.broadcast_to(g("b_out"), (128, ODIM)))
    shared = {
        "wih0T8": np.ascontiguousarray(g("W_ih_0")[_PERM].T.astype(f8)),
        "whh0T8": np.ascontiguousarray(g("W_hh_0")[_PERM].T.astype(f8)),
        "wih1T8": np.ascontiguousarray(g("W_ih_1")[_PERM].T.astype(f8)),
        "whh1T8": np.ascontiguousarray(g("W_hh_1")[_PERM].T.astype(f8)),
        "wencT": np.ascontiguousarray(g("W_enc").T.astype(f16)),
        "wdecT": np.ascontiguousarray(g("W_dec").T.astype(f16)),
        "woutT": np.ascontiguousarray(g("W_out").T.astype(f16)),
        "bg0": bg0, "bg1": bg1, "bencT": benc, "boutB": bout,
    }
    in_maps = []
    for b in range(B):
        m = dict(shared)
        m["hsT"] = np.ascontiguousarray(hs[b].T.astype(f16))
        m["eysT"] = np.ascontiguousarray(emb0[ys[b]].T.astype(f16))
        in_maps.append(m)
    return in_maps


def run(inputs, trace=False, ret_res=False):
    from concourse.bass_utils import run_bass_kernel_spmd

    if "nc" not in _CACHE:
        _CACHE["nc"] = _build_nc()
    nc = _CACHE["nc"]
    in_maps = _stage(inputs)
    res = run_bass_kernel_spmd(nc, in_maps, core_ids=list(range(B)),
                               trace=trace)
    _CACHE["last_res"] = res
    out = np.empty((B, T, U, ODIM), np.float32)
    for b in range(B):
        out[b] = res.results[b]["out"].astype(np.float32).reshape(
            U, T, ODIM).transpose(1, 0, 2)
    return out, res.exec_time_ns


def kernel(**inputs) -> np.ndarray:
    out, _ = run(inputs, trace=False)
    return out

